# revision 1
# baseline (speedup 1.0000x reference)
"""GRU unit kernel for Trainium2, data-parallel over 8 NeuronCores.

Computation (per batch row):
    r  = sigmoid(x @ W_i2r + b_i2r + h @ W_h2r)
    z  = sigmoid(x @ W_i2z + b_i2z + h @ W_h2z)
    h1 = tanh   (x @ W_i2h + b_i2h + r * (h @ W_h2h))
    out = (1 - z) * h1 + z * h

Sharding: batch (16384) split 8 ways; weights replicated.

Host-side prep: x/h transposed to [K, B_local] and cast to bf16 (so the
stationary matmul operand loads directly, no on-chip transposes), the three
x-side / h-side weight matrices concatenated to [1024, 3072] bf16, biases
concatenated and pre-broadcast to [128, 3072] f32.

Device kernel per core (B_local=2048 rows = 16 m-tiles of 128):
  - weights + xT/hT resident in SBUF (bf16), h (f32) and out streamed.
  - per m-tile: 96 bf16 matmuls of N=512 into 8 PSUM banks
    (pre_r, pre_z, x@W_i2h, h@W_h2h, each split in two 512-halves),
    fp32 accumulation; epilogue on DVE (bias adds, blend) + ACT
    (sigmoid/tanh); result DMA'd out in fp32.
"""

import os
import numpy as np
import ml_dtypes
from contextlib import ExitStack

import concourse.bass as bass
import concourse.tile as tile
from concourse import bacc, mybir

# Walrus is invoked with --enable-ldw-opt=false by default; our inner loop
# issues runs of matmuls sharing one stationary operand, so redundant
# LDWEIGHTS dominate PE overhead. Flip the flag (verified bit-identical
# output vs the reference).
if os.environ.get("GRU_LDWOPT", "0") == "1":
    import concourse.bass_utils as _bu
    if not getattr(_bu, "_gru_ldwopt_patched", False):
        _orig_run_command = _bu.run_command

        def _run_command_ldwopt(argv, **kwargs):
            argv = ["--enable-ldw-opt=true" if a == "--enable-ldw-opt=false"
                    else a for a in argv]
            return _orig_run_command(argv, **kwargs)

        _bu.run_command = _run_command_ldwopt
        _bu._gru_ldwopt_patched = True

N_CORES = 8
B, I, H = 16384, 1024, 1024
BL = B // N_CORES           # 2048 batch rows per core
MT = BL // 128              # 16 m-tiles
KO = I // 128               # 8 k-tiles of 128
F32 = mybir.dt.float32
BF16 = mybir.dt.bfloat16
BF16_NP = ml_dtypes.bfloat16


def _ap_key(a):
    try:
        return (a.memref, a.offset, str(a.ap), str(a.dtype))
    except Exception:
        return ("?", id(a))


def dedupe_ldweights(nc):
    """Drop InstLdweights that reload the stationary tile already resident in
    the PE array (bacc emits one per matmul; walrus' ldw-opt can't be used on
    these). The paired InstMatmult keeps both APs, so data deps survive; the
    removed LDW's scheduling deps are merged into the following instruction."""
    total_removed = 0
    for blk in nc.m.functions[0].blocks:
        insts = list(blk.instructions)
        new = []
        last_key = None
        pending = []
        for i in insts:
            t = type(i).__name__
            eng = str(getattr(i, "engine", ""))
            if t == "InstLdweights":
                key = (_ap_key(i.ins[0]), str(i.perf_mode),
                       str(i.tile_position), str(i.is_transpose))
                if key == last_key:
                    pending.append(i)
                    total_removed += 1
                    continue
                last_key = key
                new.append(i)
            else:
                if "PE" in eng and t not in ("InstMatmult",
                                             "InstEventSemaphore"):
                    last_key = None  # unknown PE inst may clobber weights
                if pending and t == "InstMatmult":
                    for j in pending:
                        i.merge_dependencies_from(j)
                    pending = []
                new.append(i)
        if pending:
            # dangling dup LDWs at block end (shouldn't happen) — keep them
            new.extend(pending)
        blk.instructions = new
    return total_removed


def build_nc(reps: int = 1):
    nc = bacc.Bacc("TRN2", target_bir_lowering=False, debug=False,
                   num_devices=N_CORES)
    AF = mybir.ActivationFunctionType

    xt = nc.dram_tensor("xt", [I, BL], BF16, kind="ExternalInput").ap()
    ht = nc.dram_tensor("ht", [H, BL], BF16, kind="ExternalInput").ap()
    h32 = nc.dram_tensor("h32", [BL, H], F32, kind="ExternalInput").ap()
    wx = nc.dram_tensor("wx", [I, 3 * H], BF16, kind="ExternalInput").ap()
    wh = nc.dram_tensor("wh", [H, 3 * H], BF16, kind="ExternalInput").ap()
    bias = nc.dram_tensor("bias", [128, 3 * H], F32, kind="ExternalInput").ap()
    out = nc.dram_tensor("out", [BL, H], F32, kind="ExternalOutput").ap()

    with tile.TileContext(nc) as tc, ExitStack() as ctx:
        wpool = ctx.enter_context(tc.tile_pool(name="w", bufs=1))
        apool = ctx.enter_context(tc.tile_pool(name="a", bufs=1))
        hpool = ctx.enter_context(tc.tile_pool(name="h", bufs=2))
        epool = ctx.enter_context(tc.tile_pool(name="e", bufs=3))
        psum = ctx.enter_context(tc.tile_pool(name="ps", bufs=1, space="PSUM"))

        wx_sb = wpool.tile([128, KO, 3 * H], BF16, tag="wx")
        wh_sb = wpool.tile([128, KO, 3 * H], BF16, tag="wh")
        bias_sb = wpool.tile([128, 3 * H], F32, tag="bias")
        xt_sb = apool.tile([128, KO, BL], BF16, tag="xt")
        ht_sb = apool.tile([128, KO, BL], BF16, tag="ht")

        xt_r = xt.rearrange("(ko ki) b -> ki ko b", ki=128)
        ht_r = ht.rearrange("(ko ki) b -> ki ko b", ki=128)
        wx_r = wx.rearrange("(ko ki) n -> ki ko n", ki=128)
        wh_r = wh.rearrange("(ko ki) n -> ki ko n", ki=128)

        def body():
            emit_loads()
            for mt in range(MT):
                emit_mtile(mt)

        def emit_loads():
            # Load order sets DMA FIFO order on the sync ring. m-tile 0 runs
            # gate-major (r, z, a), so feed it: xt chunk0, wx[r], wx[z],
            # wx[a], then the h-side in the same pattern.
            CH = 4
            cw = BL // CH
            nc.sync.dma_start(xt_sb[:, :, 0:cw], xt_r[:, :, 0:cw])
            for g in range(3):
                nc.sync.dma_start(wx_sb[:, :, g * H:(g + 1) * H],
                                  wx_r[:, :, g * H:(g + 1) * H])
            nc.sync.dma_start(ht_sb[:, :, 0:cw], ht_r[:, :, 0:cw])
            for g in range(3):
                nc.sync.dma_start(wh_sb[:, :, g * H:(g + 1) * H],
                                  wh_r[:, :, g * H:(g + 1) * H])
            nc.sync.dma_start(bias_sb[:], bias)
            for c in range(1, CH):
                nc.sync.dma_start(xt_sb[:, :, c * cw:(c + 1) * cw],
                                  xt_r[:, :, c * cw:(c + 1) * cw])
                nc.sync.dma_start(ht_sb[:, :, c * cw:(c + 1) * cw],
                                  ht_r[:, :, c * cw:(c + 1) * cw])

        def emit_mtile(mt):
            ms = slice(mt * 128, (mt + 1) * 128)
            h32_t = hpool.tile([128, H], F32, tag="h32")
            nc.sync.dma_start(h32_t[:], h32[ms, :])

            # 8 PSUM banks: r/z get x-side + h-side accumulated; 'a' is
            # x@W_i2h (+bias later), 'b' is h@W_h2h (scaled by r later).
            ps = {}
            for g in ("r", "z", "a", "b"):
                for nh in range(2):
                    ps[(g, nh)] = psum.tile([128, 512], F32, tag=f"p{g}{nh}",
                                            name=f"p{g}{nh}")

            # x side: gates r,z,a read wx columns [0,H),[H,2H),[2H,3H).
            # Accumulation flags: r/z span x+h sides; a is x-only, b h-only.
            def mm_x(gi, g, ko, nh):
                o = nh * 512
                nc.tensor.matmul(ps[(g, nh)], xt_sb[:, ko, ms],
                                 wx_sb[:, ko, gi * H + o:gi * H + o + 512],
                                 start=(ko == 0),
                                 stop=(g == "a" and ko == KO - 1))

            def mm_h(gi, g, ko, nh):
                o = nh * 512
                nc.tensor.matmul(ps[(g, nh)], ht_sb[:, ko, ms],
                                 wh_sb[:, ko, gi * H + o:gi * H + o + 512],
                                 start=(g == "b" and ko == 0),
                                 stop=(ko == KO - 1))

            if mt == 0:
                # Gate-major: PE can start on the first wx gate chunk instead
                # of waiting for all of wx (costs extra LDWEIGHTS, only here).
                for gi, g in enumerate(("r", "z", "a")):
                    for ko in range(KO):
                        for nh in range(2):
                            mm_x(gi, g, ko, nh)
                for gi, g in enumerate(("r", "z", "b")):
                    for ko in range(KO):
                        for nh in range(2):
                            mm_h(gi, g, ko, nh)
            else:
                # ko-major: 6 consecutive matmuls share one stationary tile,
                # deduped to one LDWEIGHTS by dedupe_ldweights(). For the
                # last m-tile, finish the 'b' banks first: the epilogue's
                # critical chain (r*b -> tanh -> blend) starts sooner.
                for ko in range(KO):
                    for nh in range(2):
                        for gi, g in enumerate(("r", "z", "a")):
                            mm_x(gi, g, ko, nh)
                if mt < MT - 1:
                    for ko in range(KO):
                        for nh in range(2):
                            for gi, g in enumerate(("r", "z", "b")):
                                mm_h(gi, g, ko, nh)
                else:
                    # Last m-tile: gate-major h-side, 'b' then 'r' then 'z',
                    # so the epilogue chain (r*b -> tanh -> z-blend) overlaps
                    # the remaining matmuls instead of serializing after them.
                    h_idx = {"r": 0, "z": 1, "b": 2}
                    for g in ("b", "r", "z"):
                        for ko in range(KO):
                            for nh in range(2):
                                mm_h(h_idx[g], g, ko, nh)

            for nh in range(2):
                o = nh * 512
                nsl = slice(o, o + 512)
                pr, pz = ps[("r", nh)], ps[("z", nh)]
                pa, pb = ps[("a", nh)], ps[("b", nh)]
                tr = epool.tile([128, 512], F32, tag="tr")
                tz = epool.tile([128, 512], F32, tag="tz")
                ta = epool.tile([128, 512], F32, tag="ta")
                nc.vector.tensor_add(tr[:], pr[:], bias_sb[:, o:o + 512])
                nc.scalar.activation(tr[:], tr[:], AF.Sigmoid)       # r
                nc.vector.tensor_add(tz[:], pz[:], bias_sb[:, H + o:H + o + 512])
                nc.scalar.activation(tz[:], tz[:], AF.Sigmoid)       # z
                nc.vector.tensor_add(ta[:], pa[:],
                                     bias_sb[:, 2 * H + o:2 * H + o + 512])
                nc.vector.tensor_mul(tr[:], tr[:], pb[:])            # r*(hU)
                nc.vector.tensor_add(ta[:], ta[:], tr[:])
                nc.scalar.activation(ta[:], ta[:], AF.Tanh)          # h1
                nc.vector.tensor_sub(tr[:], h32_t[:, nsl], ta[:])    # h-h1
                nc.vector.tensor_mul(tr[:], tz[:], tr[:])            # z*(h-h1)
                nc.vector.tensor_add(tr[:], ta[:], tr[:])            # out
                nc.scalar.dma_start(out[ms, nsl], tr[:])

        if reps > 1:
            with tc.For_i(0, reps, 1):
                body()
        else:
            body()

    nc.compile()
    if os.environ.get("GRU_DEDUP", "1") == "1":
        dedupe_ldweights(nc)
    return nc


def prep_in_maps(inputs):
    """Host-side marshalling: shard batch, transpose+cast activations,
    concat weights/biases. Returns per-core input dicts."""
    g = {k: np.asarray(v) for k, v in inputs.items()}
    x, h = g["inputs"], g["hidden"]
    wx = np.concatenate([g["W_i2r"], g["W_i2z"], g["W_i2h"]], axis=1)
    wh = np.concatenate([g["W_h2r"], g["W_h2z"], g["W_h2h"]], axis=1)
    wx = np.ascontiguousarray(wx).astype(BF16_NP)
    wh = np.ascontiguousarray(wh).astype(BF16_NP)
    b = np.concatenate([g["b_i2r"], g["b_i2z"], g["b_i2h"]]).astype(np.float32)
    bias_b = np.ascontiguousarray(np.broadcast_to(b, (128, 3 * H)))
    xt_all = x.T.astype(BF16_NP, order="C")
    ht_all = h.T.astype(BF16_NP, order="C")
    in_maps = []
    for c in range(N_CORES):
        sl = slice(c * BL, (c + 1) * BL)
        in_maps.append({
            "xt": np.ascontiguousarray(xt_all[:, sl]),
            "ht": np.ascontiguousarray(ht_all[:, sl]),
            "h32": np.ascontiguousarray(h[sl].astype(np.float32)),
            "wx": wx,
            "wh": wh,
            "bias": bias_b,
        })
    return in_maps


_RUNNERS = {}


def get_runner(reps: int = 1):
    """Build the bass module once and wrap it in a jitted 8-way shard_map,
    mirroring concourse.bass2jax.run_bass_via_pjrt but reusable across calls
    (so repeated executions don't re-trace/re-compile). reps>1 wraps the
    whole kernel in an on-device loop (for timing via amortization)."""
    if reps in _RUNNERS:
        return _RUNNERS[reps]
    import jax
    from jax.sharding import Mesh, PartitionSpec
    from jax.experimental.shard_map import shard_map
    from concourse.bass2jax import (_bass_exec_p, install_neuronx_cc_hook,
                                    partition_id_tensor)

    nc = build_nc(reps)
    install_neuronx_cc_hook()

    partition_name = (nc.partition_id_tensor.name
                      if nc.partition_id_tensor else None)
    in_names, out_names, out_avals, zero_outs = [], [], [], []
    for alloc in nc.m.functions[0].allocations:
        if not isinstance(alloc, mybir.MemoryLocationSet):
            continue
        name = alloc.memorylocations[0].name
        if alloc.kind == "ExternalInput":
            if name != partition_name:
                in_names.append(name)
        elif alloc.kind == "ExternalOutput":
            out_names.append(name)
            shape = tuple(alloc.tensor_shape)
            dtype = mybir.dt.np(alloc.dtype)
            out_avals.append(jax.core.ShapedArray(shape, dtype))
            zero_outs.append(np.zeros(shape, dtype))
    all_names = in_names + out_names
    if partition_name is not None:
        all_names = all_names + [partition_name]
    all_names = tuple(all_names)
    n_in, n_out = len(in_names), len(out_names)

    def _body(*args):
        operands = list(args)
        if partition_name is not None:
            operands.append(partition_id_tensor())
        outs = _bass_exec_p.bind(
            *operands,
            out_avals=tuple(out_avals),
            in_names=all_names,
            out_names=tuple(out_names),
            lowering_input_output_aliases=(),
            sim_require_finite=True,
            sim_require_nnan=True,
            nc=nc,
        )
        return tuple(outs)

    devices = jax.devices()[:N_CORES]
    mesh = Mesh(np.asarray(devices), ("core",))
    sharded = jax.jit(
        shard_map(_body, mesh=mesh,
                  in_specs=(PartitionSpec("core"),) * (n_in + n_out),
                  out_specs=(PartitionSpec("core"),) * n_out,
                  check_rep=False),
        donate_argnums=tuple(range(n_in, n_in + n_out)),
        keep_unused=True,
    )
    _RUNNERS[reps] = (sharded, in_names, out_names, zero_outs)
    return _RUNNERS[reps]


def run_on_device(in_maps):
    sharded, in_names, out_names, zero_outs = get_runner()
    concat_in = [np.concatenate([m[n] for m in in_maps], axis=0)
                 for n in in_names]
    concat_zero = [np.zeros((N_CORES * z.shape[0], *z.shape[1:]), z.dtype)
                   for z in zero_outs]
    outs = sharded(*concat_in, *concat_zero)
    return {n: np.asarray(o) for n, o in zip(out_names, outs)}


_NC = None


def kernel(**inputs):
    """Full-input entry point: shard, run on 8 NeuronCores, gather."""
    global _NC
    from concourse._compat import axon_active
    in_maps = prep_in_maps(inputs)
    if axon_active():
        # PJRT path with a process-cached jitted executable (repeat calls
        # skip re-trace/re-compile).
        return run_on_device(in_maps)["out"]
    from concourse.bass_utils import run_bass_kernel_spmd
    if _NC is None:
        _NC = build_nc(1)
    res = run_bass_kernel_spmd(_NC, in_maps, core_ids=list(range(N_CORES)))
    return np.concatenate([res.results[c]["out"] for c in range(N_CORES)],
                          axis=0)



# revision 8
# speedup vs baseline: 1.0134x; 1.0134x over previous
"""GRU unit kernel for Trainium2, data-parallel over 8 NeuronCores.

Computation (per batch row):
    r  = sigmoid(x @ W_i2r + b_i2r + h @ W_h2r)
    z  = sigmoid(x @ W_i2z + b_i2z + h @ W_h2z)
    h1 = tanh   (x @ W_i2h + b_i2h + r * (h @ W_h2h))
    out = (1 - z) * h1 + z * h

Sharding: batch (16384) split 8 ways; weights replicated.

Mixed-precision matmuls: per GEMM, the first F8[g] k-tiles (of 8 x 128)
are computed with fp8(e4m3) DoubleRow matmuls (2 k-tiles per pass, 2x PE
throughput), the rest with bf16 matmuls, all accumulating into the same
fp32 PSUM bank.  Inputs are pre-scaled host-side (x,h by 2^5; W by 2^12,
exact powers of two) so fp8 operands avoid the e4m3 subnormal range; the
2^-17 descale is folded into the ACT sigmoid/tanh `scale=` and the biases
are pre-scaled by 2^17.

Device kernel per core (B_local=2048 rows = 16 m-tiles of 128):
  - weights + x/h operands resident in SBUF, h (f32) and out streamed.
  - per m-tile: 8 PSUM banks (r,z,a,b x 2 N-halves of 512); stationary =
    activation tile, moving = weights; fp8 groups then bf16 groups per
    side; h-side ordered so r/b banks complete first and z last, letting
    the epilogue critical chain overlap the tail matmuls.
  - DMA rings: x-side bulk on sync, h-side bulk + h32 on gpsimd,
    outputs on scalar.
"""

import os
import numpy as np
import ml_dtypes
from contextlib import ExitStack

import concourse.bass as bass
import concourse.tile as tile
from concourse import bacc, mybir

if os.environ.get("GRU_LDWOPT", "0") == "1":
    import concourse.bass_utils as _bu
    if not getattr(_bu, "_gru_ldwopt_patched", False):
        _orig_run_command = _bu.run_command

        def _run_command_ldwopt(argv, **kwargs):
            argv = ["--enable-ldw-opt=true" if a == "--enable-ldw-opt=false"
                    else a for a in argv]
            return _orig_run_command(argv, **kwargs)

        _bu.run_command = _run_command_ldwopt
        _bu._gru_ldwopt_patched = True

N_CORES = 8
B, I, H = 16384, 1024, 1024
BL = B // N_CORES           # 2048 batch rows per core
MT = BL // 128              # 16 m-tiles
KO = I // 128               # 8 k-tiles of 128
F32 = mybir.dt.float32
BF16 = mybir.dt.bfloat16
FP8 = mybir.dt.float8e4
BF16_NP = ml_dtypes.bfloat16
FP8_NP = ml_dtypes.float8_e4m3
DR = mybir.MatmulPerfMode.DoubleRow

# fp8 k-tiles (even, 0..8) per GEMM: rx = x@W_i2r etc.
F8 = dict(
    rx=int(os.environ.get("GRU_F8_RX", "8")),
    rh=int(os.environ.get("GRU_F8_RH", "8")),
    zx=int(os.environ.get("GRU_F8_ZX", "2")),
    zh=int(os.environ.get("GRU_F8_ZH", "2")),
    ax=int(os.environ.get("GRU_F8_AX", "2")),
    bh=int(os.environ.get("GRU_F8_BH", "6")),
)

SX = 32.0                   # activation pre-scale (2^5)
SW = 4096.0                 # weight pre-scale (2^12)
ALPHA = 1.0 / (SX * SW)     # PSUM descale (2^-17)

XGATES = (("rx", 0), ("zx", 1), ("ax", 2))   # (key, column block in wx8)
HGATES = (("rh", 0), ("zh", 1), ("bh", 2))
XBANK = {"rx": "r", "zx": "z", "ax": "a"}
HBANK = {"rh": "r", "zh": "z", "bh": "b"}


def _b0(keys):
    need = [F8[k] for k in keys if F8[k] < KO]
    return min(need) if need else KO

KXB0 = _b0(["rx", "zx", "ax"])  # first k-tile with any bf16 x activation
KHB0 = _b0(["rh", "zh", "bh"])


def _ap_key(a):
    try:
        return (a.memref, a.offset, str(a.ap), str(a.dtype))
    except Exception:
        return ("?", id(a))


def dedupe_ldweights(nc):
    """Drop InstLdweights that reload the stationary tile already resident in
    the PE array (bacc emits one per matmul).  The paired InstMatmult keeps
    both APs, so data deps survive; the removed LDW's scheduling deps are
    merged into the following instruction."""
    total_removed = 0
    for blk in nc.m.functions[0].blocks:
        insts = list(blk.instructions)
        new = []
        last_key = None
        pending = []
        for i in insts:
            t = type(i).__name__
            eng = str(getattr(i, "engine", ""))
            if t == "InstLdweights":
                key = (_ap_key(i.ins[0]), str(i.perf_mode),
                       str(i.tile_position), str(i.is_transpose))
                if key == last_key:
                    pending.append(i)
                    total_removed += 1
                    continue
                last_key = key
                new.append(i)
            else:
                if "PE" in eng and t not in ("InstMatmult",
                                             "InstEventSemaphore"):
                    last_key = None  # unknown PE inst may clobber weights
                if pending and t == "InstMatmult":
                    for j in pending:
                        i.merge_dependencies_from(j)
                    pending = []
                new.append(i)
        if pending:
            new.extend(pending)
        blk.instructions = new
    return total_removed


def build_nc(reps: int = 1):
    nc = bacc.Bacc("TRN2", target_bir_lowering=False, debug=False,
                   num_devices=N_CORES)
    AF = mybir.ActivationFunctionType

    NXB = KO - KXB0             # bf16 k-tiles resident for x side
    NHB = KO - KHB0

    x8 = nc.dram_tensor("x8", [I, BL], FP8, kind="ExternalInput").ap()
    h8 = nc.dram_tensor("h8", [H, BL], FP8, kind="ExternalInput").ap()
    h32 = nc.dram_tensor("h32", [BL, H], F32, kind="ExternalInput").ap()
    wx8 = nc.dram_tensor("wx8", [I, 3 * H], FP8, kind="ExternalInput").ap()
    wh8 = nc.dram_tensor("wh8", [H, 3 * H], FP8, kind="ExternalInput").ap()
    bias = nc.dram_tensor("bias", [128, 3 * H], F32, kind="ExternalInput").ap()
    out = nc.dram_tensor("out", [BL, H], F32, kind="ExternalOutput").ap()

    # per-gate bf16 hi-k weights (exact ranges, no waste)
    wb_dram = {}
    for g in ("zx", "ax", "zh", "bh"):
        if F8[g] < KO:
            wb_dram[g] = nc.dram_tensor(
                f"wb_{g}", [(KO - F8[g]) * 128, H], BF16,
                kind="ExternalInput").ap()
    if NXB:
        xb = nc.dram_tensor("xb", [NXB * 128, BL], BF16,
                            kind="ExternalInput").ap()
    if NHB:
        hb = nc.dram_tensor("hb", [NHB * 128, BL], BF16,
                            kind="ExternalInput").ap()

    with tile.TileContext(nc) as tc, ExitStack() as ctx:
        wpool = ctx.enter_context(tc.tile_pool(name="w", bufs=1))
        apool = ctx.enter_context(tc.tile_pool(name="a", bufs=1))
        hpool = ctx.enter_context(tc.tile_pool(name="h", bufs=3))
        epool = ctx.enter_context(tc.tile_pool(name="e", bufs=2))
        psum = ctx.enter_context(tc.tile_pool(name="ps", bufs=1, space="PSUM"))

        wx8_sb = wpool.tile([128, KO, 3 * H], FP8, tag="wx8")
        wh8_sb = wpool.tile([128, KO, 3 * H], FP8, tag="wh8")
        bias_sb = wpool.tile([128, 3 * H], F32, tag="bias")
        x8_sb = apool.tile([128, KO, BL], FP8, tag="x8")
        h8_sb = apool.tile([128, KO, BL], FP8, tag="h8")
        wb_sb = {}
        for g in wb_dram:
            wb_sb[g] = wpool.tile([128, KO - F8[g], H], BF16, tag=f"wb{g}",
                                  name=f"wb{g}")
        if NXB:
            xb_sb = apool.tile([128, NXB, BL], BF16, tag="xb")
        if NHB:
            hb_sb = apool.tile([128, NHB, BL], BF16, tag="hb")

        x8_r = x8.rearrange("(ko ki) b -> ki ko b", ki=128)
        h8_r = h8.rearrange("(ko ki) b -> ki ko b", ki=128)
        wx8_r = wx8.rearrange("(ko ki) n -> ki ko n", ki=128)
        wh8_r = wh8.rearrange("(ko ki) n -> ki ko n", ki=128)
        wb_r = {g: wb_dram[g].rearrange("(ko ki) n -> ki ko n", ki=128)
                for g in wb_dram}
        if NXB:
            xb_r = xb.rearrange("(ko ki) b -> ki ko b", ki=128)
        if NHB:
            hb_r = hb.rearrange("(ko ki) b -> ki ko b", ki=128)

        h32_t = [None] * MT

        def h32_load(mt):
            if h32_t[mt] is None:
                h32_t[mt] = hpool.tile([128, H], F32, tag="h32", name="h32")
                nc.gpsimd.dma_start(h32_t[mt][:],
                                    h32[mt * 128:(mt + 1) * 128, :])

        def body():
            emit_loads()
            for mt in range(MT):
                emit_mtile(mt)
                h32_t[mt] = None

        def emit_loads():
            # FIFO order per ring == consumption order of m-tile 0.
            CH = 4
            cw = BL // CH
            c0 = slice(0, cw)
            nc.sync.dma_start(x8_sb[:, :, c0], x8_r[:, :, c0])
            for g, gi in XGATES:
                if F8[g]:
                    nc.sync.dma_start(
                        wx8_sb[:, 0:F8[g], gi * H:(gi + 1) * H],
                        wx8_r[:, 0:F8[g], gi * H:(gi + 1) * H])
            if NXB:
                nc.sync.dma_start(xb_sb[:, :, c0], xb_r[:, :, c0])
            for g in ("zx", "ax"):
                if g in wb_sb:
                    nc.sync.dma_start(wb_sb[g][:], wb_r[g])
            nc.gpsimd.dma_start(h8_sb[:, :, c0], h8_r[:, :, c0])
            for g, gi in HGATES:
                if F8[g]:
                    nc.gpsimd.dma_start(
                        wh8_sb[:, 0:F8[g], gi * H:(gi + 1) * H],
                        wh8_r[:, 0:F8[g], gi * H:(gi + 1) * H])
            if NHB:
                nc.gpsimd.dma_start(hb_sb[:, :, c0], hb_r[:, :, c0])
            for g in ("bh", "zh"):
                if g in wb_sb:
                    nc.gpsimd.dma_start(wb_sb[g][:], wb_r[g])
            nc.sync.dma_start(bias_sb[:], bias)
            for mt in range(4):
                h32_load(mt)
            for c in range(1, CH):
                cs = slice(c * cw, (c + 1) * cw)
                nc.sync.dma_start(x8_sb[:, :, cs], x8_r[:, :, cs])
                nc.gpsimd.dma_start(h8_sb[:, :, cs], h8_r[:, :, cs])
                if NXB:
                    nc.sync.dma_start(xb_sb[:, :, cs], xb_r[:, :, cs])
                if NHB:
                    nc.gpsimd.dma_start(hb_sb[:, :, cs], hb_r[:, :, cs])
                for mt in range(4 * c, 4 * c + 4):
                    h32_load(mt)

        def emit_mtile(mt):
            ms = slice(mt * 128, (mt + 1) * 128)
            h32_load(mt)

            ps = {}
            for g in ("r", "z", "a", "b"):
                for nh in range(2):
                    ps[(g, nh)] = psum.tile([128, 512], F32, tag=f"p{g}{nh}",
                                            name=f"p{g}{nh}")
            started = set()

            def _passes(key):
                return F8[key] // 2 + (KO - F8[key])

            left = {
                "r": _passes("rx") + _passes("rh"),
                "z": _passes("zx") + _passes("zh"),
                "a": _passes("ax"),
                "b": _passes("bh"),
            }
            rem = {(g, nh): left[g] for g in left for nh in range(2)}

            def mm(bank, nh, stat, mov, perf_mode):
                key = (bank, nh)
                start = key not in started
                started.add(key)
                rem[key] -= 1
                nc.tensor.matmul(ps[key], stat, mov, start=start,
                                 stop=(rem[key] == 0), perf_mode=perf_mode)

            # ---- x side: fp8 DoubleRow groups (stationary = x8 tile) ----
            for kp in range(0, max(F8[g] for g, _ in XGATES), 2):
                stat = x8_sb[:, kp:kp + 2, ms]
                for nh in range(2):
                    o = nh * 512
                    for g, gi in XGATES:
                        if F8[g] > kp:
                            mm(XBANK[g], nh, stat,
                               wx8_sb[:, kp:kp + 2,
                                      gi * H + o:gi * H + o + 512], DR)
            # ---- x side: bf16 groups ----
            for ko in range(KXB0, KO):
                stat = xb_sb[:, ko - KXB0, ms]
                for nh in range(2):
                    o = nh * 512
                    for g in ("zx", "ax"):
                        if F8[g] <= ko:
                            mm(XBANK[g], nh, stat,
                               wb_sb[g][:, ko - F8[g], o:o + 512], None)
            # ---- h side: fp8 groups ----
            for kp in range(0, max(F8[g] for g, _ in HGATES), 2):
                stat = h8_sb[:, kp:kp + 2, ms]
                for nh in range(2):
                    o = nh * 512
                    for g, gi in HGATES:
                        if F8[g] > kp:
                            mm(HBANK[g], nh, stat,
                               wh8_sb[:, kp:kp + 2,
                                      gi * H + o:gi * H + o + 512], DR)
            # ---- h side: bf16 groups, b's k-tiles before z's ----
            for g in ("bh", "zh"):
                for ko in range(F8[g], KO):
                    stat = hb_sb[:, ko - KHB0, ms]
                    for nh in range(2):
                        o = nh * 512
                        mm(HBANK[g], nh, stat,
                           wb_sb[g][:, ko - F8[g], o:o + 512], None)

            # ---- epilogue ----
            for nh in range(2):
                o = nh * 512
                nsl = slice(o, o + 512)
                pr, pz = ps[("r", nh)], ps[("z", nh)]
                pa, pb = ps[("a", nh)], ps[("b", nh)]
                tr = epool.tile([128, 512], F32, tag="tr")
                tz = epool.tile([128, 512], F32, tag="tz")
                ta = epool.tile([128, 512], F32, tag="ta")
                nc.vector.tensor_add(tr[:], pr[:], bias_sb[:, o:o + 512])
                nc.scalar.activation(tr[:], tr[:], AF.Sigmoid, scale=ALPHA)
                nc.vector.tensor_mul(tr[:], tr[:], pb[:])            # r*(hU)
                nc.vector.tensor_add(ta[:], pa[:],
                                     bias_sb[:, 2 * H + o:2 * H + o + 512])
                nc.vector.tensor_add(ta[:], ta[:], tr[:])
                nc.scalar.activation(ta[:], ta[:], AF.Tanh, scale=ALPHA)
                nc.vector.tensor_add(tz[:], pz[:],
                                     bias_sb[:, H + o:H + o + 512])
                nc.scalar.activation(tz[:], tz[:], AF.Sigmoid, scale=ALPHA)
                nc.vector.tensor_sub(tr[:], h32_t[mt][:, nsl], ta[:])
                nc.vector.tensor_mul(tr[:], tz[:], tr[:])            # z*(h-h1)
                nc.vector.tensor_add(tr[:], ta[:], tr[:])            # out
                nc.scalar.dma_start(out[ms, nsl], tr[:])

        if reps > 1:
            with tc.For_i(0, reps, 1):
                body()
        else:
            body()

    nc.compile()
    if os.environ.get("GRU_DEDUP", "1") == "1":
        dedupe_ldweights(nc)
    return nc


def prep_in_maps(inputs):
    """Host-side marshalling: shard batch, transpose/scale/quantize
    activations, concat weights/biases.  Returns per-core input dicts."""
    g = {k: np.asarray(v) for k, v in inputs.items()}
    x, h = g["inputs"].astype(np.float32), g["hidden"].astype(np.float32)
    wx = np.concatenate([g["W_i2r"], g["W_i2z"], g["W_i2h"]],
                        axis=1).astype(np.float32) * SW
    wh = np.concatenate([g["W_h2r"], g["W_h2z"], g["W_h2h"]],
                        axis=1).astype(np.float32) * SW
    b = np.concatenate([g["b_i2r"], g["b_i2z"], g["b_i2h"]]).astype(np.float32)
    bias_b = np.ascontiguousarray(np.broadcast_to(b / ALPHA, (128, 3 * H)))

    xs = np.ascontiguousarray(x.T) * SX          # [I, B], scaled
    hs = np.ascontiguousarray(h.T) * SX
    x8_all = xs.astype(FP8_NP)
    h8_all = hs.astype(FP8_NP)
    wx8 = np.ascontiguousarray(wx).astype(FP8_NP)
    wh8 = np.ascontiguousarray(wh).astype(FP8_NP)

    wcol = {"zx": wx[:, H:2 * H], "ax": wx[:, 2 * H:3 * H],
            "zh": wh[:, H:2 * H], "bh": wh[:, 2 * H:3 * H]}
    wb = {}
    for gkey in ("zx", "ax", "zh", "bh"):
        if F8[gkey] < KO:
            wb[gkey] = np.ascontiguousarray(
                wcol[gkey][F8[gkey] * 128:]).astype(BF16_NP)

    NXB = KO - KXB0
    NHB = KO - KHB0
    if NXB:
        xb_all = np.ascontiguousarray(xs[KXB0 * 128:]).astype(BF16_NP)
    if NHB:
        hb_all = np.ascontiguousarray(hs[KHB0 * 128:]).astype(BF16_NP)

    in_maps = []
    for c in range(N_CORES):
        sl = slice(c * BL, (c + 1) * BL)
        m = {
            "x8": np.ascontiguousarray(x8_all[:, sl]),
            "h8": np.ascontiguousarray(h8_all[:, sl]),
            "h32": np.ascontiguousarray(h[sl]),
            "wx8": wx8,
            "wh8": wh8,
            "bias": bias_b,
        }
        for gkey, arr in wb.items():
            m[f"wb_{gkey}"] = arr
        if NXB:
            m["xb"] = np.ascontiguousarray(xb_all[:, sl])
        if NHB:
            m["hb"] = np.ascontiguousarray(hb_all[:, sl])
        in_maps.append(m)
    return in_maps


_RUNNERS = {}


def get_runner(reps: int = 1):
    """Build the bass module once and wrap it in a jitted 8-way shard_map
    (so repeated executions don't re-trace/re-compile).  reps>1 wraps the
    whole kernel in an on-device loop (for timing via amortization)."""
    if reps in _RUNNERS:
        return _RUNNERS[reps]
    import jax
    from jax.sharding import Mesh, PartitionSpec
    from jax.experimental.shard_map import shard_map
    from concourse.bass2jax import (_bass_exec_p, install_neuronx_cc_hook,
                                    partition_id_tensor)

    nc = build_nc(reps)
    install_neuronx_cc_hook()

    partition_name = (nc.partition_id_tensor.name
                      if nc.partition_id_tensor else None)
    in_names, out_names, out_avals, zero_outs = [], [], [], []
    for alloc in nc.m.functions[0].allocations:
        if not isinstance(alloc, mybir.MemoryLocationSet):
            continue
        name = alloc.memorylocations[0].name
        if alloc.kind == "ExternalInput":
            if name != partition_name:
                in_names.append(name)
        elif alloc.kind == "ExternalOutput":
            out_names.append(name)
            shape = tuple(alloc.tensor_shape)
            dtype = mybir.dt.np(alloc.dtype)
            out_avals.append(jax.core.ShapedArray(shape, dtype))
            zero_outs.append(np.zeros(shape, dtype))
    all_names = in_names + out_names
    if partition_name is not None:
        all_names = all_names + [partition_name]
    all_names = tuple(all_names)
    n_in, n_out = len(in_names), len(out_names)

    def _body(*args):
        operands = list(args)
        if partition_name is not None:
            operands.append(partition_id_tensor())
        outs = _bass_exec_p.bind(
            *operands,
            out_avals=tuple(out_avals),
            in_names=all_names,
            out_names=tuple(out_names),
            lowering_input_output_aliases=(),
            sim_require_finite=True,
            sim_require_nnan=True,
            nc=nc,
        )
        return tuple(outs)

    devices = jax.devices()[:N_CORES]
    mesh = Mesh(np.asarray(devices), ("core",))
    sharded = jax.jit(
        shard_map(_body, mesh=mesh,
                  in_specs=(PartitionSpec("core"),) * (n_in + n_out),
                  out_specs=(PartitionSpec("core"),) * n_out,
                  check_rep=False),
        donate_argnums=tuple(range(n_in, n_in + n_out)),
        keep_unused=True,
    )
    _RUNNERS[reps] = (sharded, in_names, out_names, zero_outs)
    return _RUNNERS[reps]


def run_on_device(in_maps):
    sharded, in_names, out_names, zero_outs = get_runner()
    concat_in = [np.concatenate([m[n] for m in in_maps], axis=0)
                 for n in in_names]
    concat_zero = [np.zeros((N_CORES * z.shape[0], *z.shape[1:]), z.dtype)
                   for z in zero_outs]
    outs = sharded(*concat_in, *concat_zero)
    return {n: np.asarray(o) for n, o in zip(out_names, outs)}


_NC = None


def kernel(**inputs):
    """Full-input entry point: shard, run on 8 NeuronCores, gather."""
    global _NC
    from concourse._compat import axon_active
    in_maps = prep_in_maps(inputs)
    if axon_active():
        return run_on_device(in_maps)["out"]
    from concourse.bass_utils import run_bass_kernel_spmd
    if _NC is None:
        _NC = build_nc(1)
    res = run_bass_kernel_spmd(_NC, in_maps, core_ids=list(range(N_CORES)))
    return np.concatenate([res.results[c]["out"] for c in range(N_CORES)],
                          axis=0)


# revision 12
# speedup vs baseline: 1.3171x; 1.2997x over previous
"""GRU unit kernel for Trainium2, data-parallel over 8 NeuronCores.

Computation (per batch row):
    r  = sigmoid(x @ W_i2r + b_i2r + h @ W_h2r)
    z  = sigmoid(x @ W_i2z + b_i2z + h @ W_h2z)
    h1 = tanh   (x @ W_i2h + b_i2h + r * (h @ W_h2h))
    out = (1 - z) * h1 + z * h

Sharding: batch (16384) split 8 ways; weights replicated.

Mixed-precision matmuls: per GEMM, the first F8[g] k-tiles (of 8 x 128)
are computed with fp8(e4m3) DoubleRow matmuls (2 k-tiles per pass, 2x PE
throughput), the rest with bf16 matmuls, all accumulating into the same
fp32 PSUM bank.  Inputs are pre-scaled host-side (x,h by 2^5; W by 2^12,
exact powers of two) so fp8 operands avoid the e4m3 subnormal range; the
2^-17 descale is folded into the ACT sigmoid/tanh `scale=` and the biases
are pre-scaled by 2^17.

Device kernel per core (B_local=2048 rows = 16 m-tiles of 128):
  - weights + x/h operands resident in SBUF, h (f32) and out streamed.
  - per m-tile: 8 PSUM banks (r,z,a,b x 2 N-halves of 512); stationary =
    activation tile, moving = weights; fp8 groups then bf16 groups per
    side; h-side ordered so r/b banks complete first and z last, letting
    the epilogue critical chain overlap the tail matmuls.
  - DMA rings: x-side bulk on sync, h-side bulk + h32 on gpsimd,
    outputs on scalar.
"""

import os
import numpy as np
import ml_dtypes
from contextlib import ExitStack

import concourse.bass as bass
import concourse.tile as tile
from concourse import bacc, mybir

if os.environ.get("GRU_LDWOPT", "0") == "1":
    import concourse.bass_utils as _bu
    if not getattr(_bu, "_gru_ldwopt_patched", False):
        _orig_run_command = _bu.run_command

        def _run_command_ldwopt(argv, **kwargs):
            argv = ["--enable-ldw-opt=true" if a == "--enable-ldw-opt=false"
                    else a for a in argv]
            return _orig_run_command(argv, **kwargs)

        _bu.run_command = _run_command_ldwopt
        _bu._gru_ldwopt_patched = True

N_CORES = 8
B, I, H = 16384, 1024, 1024
BL = B // N_CORES           # 2048 batch rows per core
MT = BL // 128              # 16 m-tiles
KO = I // 128               # 8 k-tiles of 128
F32 = mybir.dt.float32
BF16 = mybir.dt.bfloat16
FP8 = mybir.dt.float8e4
BF16_NP = ml_dtypes.bfloat16
FP8_NP = ml_dtypes.float8_e4m3
DR = mybir.MatmulPerfMode.DoubleRow

# fp8 k-tiles (even, 0..8) per GEMM: rx = x@W_i2r etc.
F8 = dict(
    rx=int(os.environ.get("GRU_F8_RX", "8")),
    rh=int(os.environ.get("GRU_F8_RH", "8")),
    zx=int(os.environ.get("GRU_F8_ZX", "2")),
    zh=int(os.environ.get("GRU_F8_ZH", "2")),
    ax=int(os.environ.get("GRU_F8_AX", "2")),
    bh=int(os.environ.get("GRU_F8_BH", "6")),
)

EPOFF = os.environ.get("GRU_EPOFF", "0") == "1"   # timing-only: no epilogue
KPORD = os.environ.get("GRU_KPORD", "1") == "1"   # r-only kp groups first

SX = 32.0                   # activation pre-scale (2^5)
SW = 4096.0                 # weight pre-scale (2^12)
ALPHA = 1.0 / (SX * SW)     # PSUM descale (2^-17)

XGATES = (("rx", 0), ("zx", 1), ("ax", 2))   # (key, column block in wx8)
HGATES = (("rh", 0), ("zh", 1), ("bh", 2))
XBANK = {"rx": "r", "zx": "z", "ax": "a"}
HBANK = {"rh": "r", "zh": "z", "bh": "b"}


def _b0(keys):
    need = [F8[k] for k in keys if F8[k] < KO]
    return min(need) if need else KO

KXB0 = _b0(["rx", "zx", "ax"])  # first k-tile with any bf16 x activation
KHB0 = _b0(["rh", "zh", "bh"])


def _ap_key(a):
    try:
        return (a.memref, a.offset, str(a.ap), str(a.dtype))
    except Exception:
        return ("?", id(a))


def dedupe_ldweights(nc):
    """Drop InstLdweights that reload the stationary tile already resident in
    the PE array (bacc emits one per matmul).  The paired InstMatmult keeps
    both APs, so data deps survive; the removed LDW's scheduling deps are
    merged into the following instruction."""
    total_removed = 0
    for blk in nc.m.functions[0].blocks:
        insts = list(blk.instructions)
        new = []
        last_key = None
        pending = []
        for i in insts:
            t = type(i).__name__
            eng = str(getattr(i, "engine", ""))
            if t == "InstLdweights":
                key = (_ap_key(i.ins[0]), str(i.perf_mode),
                       str(i.tile_position), str(i.is_transpose))
                if key == last_key:
                    pending.append(i)
                    total_removed += 1
                    continue
                last_key = key
                new.append(i)
            else:
                if "PE" in eng and t not in ("InstMatmult",
                                             "InstEventSemaphore"):
                    last_key = None  # unknown PE inst may clobber weights
                if pending and t == "InstMatmult":
                    for j in pending:
                        i.merge_dependencies_from(j)
                    pending = []
                new.append(i)
        if pending:
            new.extend(pending)
        blk.instructions = new
    return total_removed


def build_nc(reps: int = 1):
    nc = bacc.Bacc("TRN2", target_bir_lowering=False, debug=False,
                   num_devices=N_CORES)
    AF = mybir.ActivationFunctionType

    NXB = KO - KXB0             # bf16 k-tiles resident for x side
    NHB = KO - KHB0

    x8 = nc.dram_tensor("x8", [I, BL], FP8, kind="ExternalInput").ap()
    h8 = nc.dram_tensor("h8", [H, BL], FP8, kind="ExternalInput").ap()
    h32 = nc.dram_tensor("h32", [BL, H], F32, kind="ExternalInput").ap()
    wx8 = nc.dram_tensor("wx8", [I, 3 * H], FP8, kind="ExternalInput").ap()
    wh8 = nc.dram_tensor("wh8", [H, 3 * H], FP8, kind="ExternalInput").ap()
    bias = nc.dram_tensor("bias", [128, 3 * H], F32, kind="ExternalInput").ap()
    out = nc.dram_tensor("out", [BL, H], F32, kind="ExternalOutput").ap()

    # per-gate bf16 hi-k weights (exact ranges, no waste)
    wb_dram = {}
    for g in ("zx", "ax", "zh", "bh"):
        if F8[g] < KO:
            wb_dram[g] = nc.dram_tensor(
                f"wb_{g}", [(KO - F8[g]) * 128, H], BF16,
                kind="ExternalInput").ap()
    if NXB:
        xb = nc.dram_tensor("xb", [NXB * 128, BL], BF16,
                            kind="ExternalInput").ap()
    if NHB:
        hb = nc.dram_tensor("hb", [NHB * 128, BL], BF16,
                            kind="ExternalInput").ap()

    with tile.TileContext(nc) as tc, ExitStack() as ctx:
        wpool = ctx.enter_context(tc.tile_pool(name="w", bufs=1))
        apool = ctx.enter_context(tc.tile_pool(name="a", bufs=1))
        hpool = ctx.enter_context(tc.tile_pool(name="h", bufs=3))
        epool = ctx.enter_context(tc.tile_pool(name="e", bufs=2))
        psum = ctx.enter_context(tc.tile_pool(name="ps", bufs=1, space="PSUM"))

        wx8_sb = wpool.tile([128, KO, 3 * H], FP8, tag="wx8")
        wh8_sb = wpool.tile([128, KO, 3 * H], FP8, tag="wh8")
        bias_sb = wpool.tile([128, 3 * H], F32, tag="bias")
        x8_sb = apool.tile([128, KO, BL], FP8, tag="x8")
        h8_sb = apool.tile([128, KO, BL], FP8, tag="h8")
        wb_sb = {}
        for g in wb_dram:
            wb_sb[g] = wpool.tile([128, KO - F8[g], H], BF16, tag=f"wb{g}",
                                  name=f"wb{g}")
        if NXB:
            xb_sb = apool.tile([128, NXB, BL], BF16, tag="xb")
        if NHB:
            hb_sb = apool.tile([128, NHB, BL], BF16, tag="hb")

        x8_r = x8.rearrange("(ko ki) b -> ki ko b", ki=128)
        h8_r = h8.rearrange("(ko ki) b -> ki ko b", ki=128)
        wx8_r = wx8.rearrange("(ko ki) n -> ki ko n", ki=128)
        wh8_r = wh8.rearrange("(ko ki) n -> ki ko n", ki=128)
        wb_r = {g: wb_dram[g].rearrange("(ko ki) n -> ki ko n", ki=128)
                for g in wb_dram}
        if NXB:
            xb_r = xb.rearrange("(ko ki) b -> ki ko b", ki=128)
        if NHB:
            hb_r = hb.rearrange("(ko ki) b -> ki ko b", ki=128)

        h32_t = [None] * MT

        def h32_load(mt):
            if EPOFF:
                return
            if h32_t[mt] is None:
                h32_t[mt] = hpool.tile([128, H], F32, tag="h32", name="h32")
                nc.gpsimd.dma_start(h32_t[mt][:],
                                    h32[mt * 128:(mt + 1) * 128, :])

        def body():
            emit_loads()
            for mt in range(MT):
                emit_mtile(mt)
                h32_t[mt] = None

        def emit_loads():
            # FIFO order per ring == consumption order of m-tile 0.
            CH = 4
            cw = BL // CH
            c0 = slice(0, cw)
            nc.sync.dma_start(x8_sb[:, :, c0], x8_r[:, :, c0])
            for g, gi in XGATES:
                if F8[g]:
                    nc.sync.dma_start(
                        wx8_sb[:, 0:F8[g], gi * H:(gi + 1) * H],
                        wx8_r[:, 0:F8[g], gi * H:(gi + 1) * H])
            if NXB:
                nc.sync.dma_start(xb_sb[:, :, c0], xb_r[:, :, c0])
            for g in ("zx", "ax"):
                if g in wb_sb:
                    nc.sync.dma_start(wb_sb[g][:], wb_r[g])
            nc.gpsimd.dma_start(h8_sb[:, :, c0], h8_r[:, :, c0])
            for g, gi in HGATES:
                if F8[g]:
                    nc.gpsimd.dma_start(
                        wh8_sb[:, 0:F8[g], gi * H:(gi + 1) * H],
                        wh8_r[:, 0:F8[g], gi * H:(gi + 1) * H])
            if NHB:
                nc.gpsimd.dma_start(hb_sb[:, :, c0], hb_r[:, :, c0])
            for g in ("bh", "zh"):
                if g in wb_sb:
                    nc.gpsimd.dma_start(wb_sb[g][:], wb_r[g])
            nc.sync.dma_start(bias_sb[:], bias)
            for mt in range(4):
                h32_load(mt)
            for c in range(1, CH):
                cs = slice(c * cw, (c + 1) * cw)
                nc.sync.dma_start(x8_sb[:, :, cs], x8_r[:, :, cs])
                nc.gpsimd.dma_start(h8_sb[:, :, cs], h8_r[:, :, cs])
                if NXB:
                    nc.sync.dma_start(xb_sb[:, :, cs], xb_r[:, :, cs])
                if NHB:
                    nc.gpsimd.dma_start(hb_sb[:, :, cs], hb_r[:, :, cs])
                for mt in range(4 * c, 4 * c + 4):
                    h32_load(mt)

        def emit_mtile(mt):
            ms = slice(mt * 128, (mt + 1) * 128)
            h32_load(mt)

            ps = {}
            for g in ("r", "z", "a", "b"):
                for nh in range(2):
                    ps[(g, nh)] = psum.tile([128, 512], F32, tag=f"p{g}{nh}",
                                            name=f"p{g}{nh}")
            started = set()

            def _passes(key):
                return F8[key] // 2 + (KO - F8[key])

            left = {
                "r": _passes("rx") + _passes("rh"),
                "z": _passes("zx") + _passes("zh"),
                "a": _passes("ax"),
                "b": _passes("bh"),
            }
            rem = {(g, nh): left[g] for g in left for nh in range(2)}

            def mm(bank, nh, stat, mov, perf_mode):
                key = (bank, nh)
                start = key not in started
                started.add(key)
                rem[key] -= 1
                nc.tensor.matmul(ps[key], stat, mov, start=start,
                                 stop=(rem[key] == 0), perf_mode=perf_mode)

            # ---- x side: fp8 DoubleRow groups (stationary = x8 tile) ----
            # r-only kp groups first: the m-tile's first matmuls then touch
            # only the r bank, which the previous epilogue frees earliest.
            kps = list(range(0, max(F8[g] for g, _ in XGATES), 2))
            if KPORD:
                kps.sort(key=lambda kp: sum(1 for g, _ in XGATES[1:]
                                            if F8[g] > kp))
            for kp in kps:
                stat = x8_sb[:, kp:kp + 2, ms]
                for nh in range(2):
                    o = nh * 512
                    for g, gi in XGATES:
                        if F8[g] > kp:
                            mm(XBANK[g], nh, stat,
                               wx8_sb[:, kp:kp + 2,
                                      gi * H + o:gi * H + o + 512], DR)
            # ---- x side: bf16 groups ----
            for ko in range(KXB0, KO):
                stat = xb_sb[:, ko - KXB0, ms]
                for nh in range(2):
                    o = nh * 512
                    for g in ("zx", "ax"):
                        if F8[g] <= ko:
                            mm(XBANK[g], nh, stat,
                               wb_sb[g][:, ko - F8[g], o:o + 512], None)
            # ---- h side: fp8 groups ----
            for kp in range(0, max(F8[g] for g, _ in HGATES), 2):
                stat = h8_sb[:, kp:kp + 2, ms]
                for nh in range(2):
                    o = nh * 512
                    for g, gi in HGATES:
                        if F8[g] > kp:
                            mm(HBANK[g], nh, stat,
                               wh8_sb[:, kp:kp + 2,
                                      gi * H + o:gi * H + o + 512], DR)
            # ---- h side: bf16 groups, b's k-tiles before z's ----
            for g in ("bh", "zh"):
                for ko in range(F8[g], KO):
                    stat = hb_sb[:, ko - KHB0, ms]
                    for nh in range(2):
                        o = nh * 512
                        mm(HBANK[g], nh, stat,
                           wb_sb[g][:, ko - F8[g], o:o + 512], None)

            if EPOFF:
                for nh in range(2):
                    o = nh * 512
                    sc = epool.tile([128, 512], F32, tag="tr", name="sc")
                    for g in ("r", "z", "a", "b"):
                        nc.vector.tensor_copy(sc[:], ps[(g, nh)][:])
                    nc.scalar.dma_start(out[ms, o:o + 512], sc[:])
                return

            # ---- epilogue ----
            for nh in range(2):
                o = nh * 512
                nsl = slice(o, o + 512)
                pr, pz = ps[("r", nh)], ps[("z", nh)]
                pa, pb = ps[("a", nh)], ps[("b", nh)]
                tr = epool.tile([128, 512], F32, tag="tr")
                tz = epool.tile([128, 512], F32, tag="tz")
                ta = epool.tile([128, 512], F32, tag="ta")
                nc.vector.tensor_add(tr[:], pr[:], bias_sb[:, o:o + 512])
                nc.scalar.activation(tr[:], tr[:], AF.Sigmoid, scale=ALPHA)
                nc.vector.tensor_mul(tr[:], tr[:], pb[:])            # r*(hU)
                nc.vector.tensor_add(ta[:], pa[:],
                                     bias_sb[:, 2 * H + o:2 * H + o + 512])
                nc.vector.tensor_add(ta[:], ta[:], tr[:])
                nc.scalar.activation(ta[:], ta[:], AF.Tanh, scale=ALPHA)
                nc.vector.tensor_add(tz[:], pz[:],
                                     bias_sb[:, H + o:H + o + 512])
                nc.scalar.activation(tz[:], tz[:], AF.Sigmoid, scale=ALPHA)
                nc.vector.tensor_sub(tr[:], h32_t[mt][:, nsl], ta[:])
                nc.vector.tensor_mul(tr[:], tz[:], tr[:])            # z*(h-h1)
                nc.vector.tensor_add(tr[:], ta[:], tr[:])            # out
                nc.scalar.dma_start(out[ms, nsl], tr[:])

        if reps > 1:
            with tc.For_i(0, reps, 1):
                body()
        else:
            body()

    nc.compile()
    if os.environ.get("GRU_DEDUP", "1") == "1":
        dedupe_ldweights(nc)
    return nc


def prep_in_maps(inputs):
    """Host-side marshalling: shard batch, transpose/scale/quantize
    activations, concat weights/biases.  Returns per-core input dicts."""
    g = {k: np.asarray(v) for k, v in inputs.items()}
    x, h = g["inputs"].astype(np.float32), g["hidden"].astype(np.float32)
    wx = np.concatenate([g["W_i2r"], g["W_i2z"], g["W_i2h"]],
                        axis=1).astype(np.float32) * SW
    wh = np.concatenate([g["W_h2r"], g["W_h2z"], g["W_h2h"]],
                        axis=1).astype(np.float32) * SW
    b = np.concatenate([g["b_i2r"], g["b_i2z"], g["b_i2h"]]).astype(np.float32)
    bias_b = np.ascontiguousarray(np.broadcast_to(b / ALPHA, (128, 3 * H)))

    xs = np.ascontiguousarray(x.T) * SX          # [I, B], scaled
    hs = np.ascontiguousarray(h.T) * SX
    x8_all = xs.astype(FP8_NP)
    h8_all = hs.astype(FP8_NP)
    wx8 = np.ascontiguousarray(wx).astype(FP8_NP)
    wh8 = np.ascontiguousarray(wh).astype(FP8_NP)

    wcol = {"zx": wx[:, H:2 * H], "ax": wx[:, 2 * H:3 * H],
            "zh": wh[:, H:2 * H], "bh": wh[:, 2 * H:3 * H]}
    wb = {}
    for gkey in ("zx", "ax", "zh", "bh"):
        if F8[gkey] < KO:
            wb[gkey] = np.ascontiguousarray(
                wcol[gkey][F8[gkey] * 128:]).astype(BF16_NP)

    NXB = KO - KXB0
    NHB = KO - KHB0
    if NXB:
        xb_all = np.ascontiguousarray(xs[KXB0 * 128:]).astype(BF16_NP)
    if NHB:
        hb_all = np.ascontiguousarray(hs[KHB0 * 128:]).astype(BF16_NP)

    in_maps = []
    for c in range(N_CORES):
        sl = slice(c * BL, (c + 1) * BL)
        m = {
            "x8": np.ascontiguousarray(x8_all[:, sl]),
            "h8": np.ascontiguousarray(h8_all[:, sl]),
            "h32": np.ascontiguousarray(h[sl]),
            "wx8": wx8,
            "wh8": wh8,
            "bias": bias_b,
        }
        for gkey, arr in wb.items():
            m[f"wb_{gkey}"] = arr
        if NXB:
            m["xb"] = np.ascontiguousarray(xb_all[:, sl])
        if NHB:
            m["hb"] = np.ascontiguousarray(hb_all[:, sl])
        in_maps.append(m)
    return in_maps


_RUNNERS = {}


def get_runner(reps: int = 1):
    """Build the bass module once and wrap it in a jitted 8-way shard_map
    (so repeated executions don't re-trace/re-compile).  reps>1 wraps the
    whole kernel in an on-device loop (for timing via amortization)."""
    if reps in _RUNNERS:
        return _RUNNERS[reps]
    import jax
    from jax.sharding import Mesh, PartitionSpec
    from jax.experimental.shard_map import shard_map
    from concourse.bass2jax import (_bass_exec_p, install_neuronx_cc_hook,
                                    partition_id_tensor)

    nc = build_nc(reps)
    install_neuronx_cc_hook()

    partition_name = (nc.partition_id_tensor.name
                      if nc.partition_id_tensor else None)
    in_names, out_names, out_avals, zero_outs = [], [], [], []
    for alloc in nc.m.functions[0].allocations:
        if not isinstance(alloc, mybir.MemoryLocationSet):
            continue
        name = alloc.memorylocations[0].name
        if alloc.kind == "ExternalInput":
            if name != partition_name:
                in_names.append(name)
        elif alloc.kind == "ExternalOutput":
            out_names.append(name)
            shape = tuple(alloc.tensor_shape)
            dtype = mybir.dt.np(alloc.dtype)
            out_avals.append(jax.core.ShapedArray(shape, dtype))
            zero_outs.append(np.zeros(shape, dtype))
    all_names = in_names + out_names
    if partition_name is not None:
        all_names = all_names + [partition_name]
    all_names = tuple(all_names)
    n_in, n_out = len(in_names), len(out_names)

    def _body(*args):
        operands = list(args)
        if partition_name is not None:
            operands.append(partition_id_tensor())
        outs = _bass_exec_p.bind(
            *operands,
            out_avals=tuple(out_avals),
            in_names=all_names,
            out_names=tuple(out_names),
            lowering_input_output_aliases=(),
            sim_require_finite=True,
            sim_require_nnan=True,
            nc=nc,
        )
        return tuple(outs)

    devices = jax.devices()[:N_CORES]
    mesh = Mesh(np.asarray(devices), ("core",))
    sharded = jax.jit(
        shard_map(_body, mesh=mesh,
                  in_specs=(PartitionSpec("core"),) * (n_in + n_out),
                  out_specs=(PartitionSpec("core"),) * n_out,
                  check_rep=False),
        donate_argnums=tuple(range(n_in, n_in + n_out)),
        keep_unused=True,
    )
    _RUNNERS[reps] = (sharded, in_names, out_names, zero_outs)
    return _RUNNERS[reps]


def run_on_device(in_maps):
    sharded, in_names, out_names, zero_outs = get_runner()
    concat_in = [np.concatenate([m[n] for m in in_maps], axis=0)
                 for n in in_names]
    concat_zero = [np.zeros((N_CORES * z.shape[0], *z.shape[1:]), z.dtype)
                   for z in zero_outs]
    outs = sharded(*concat_in, *concat_zero)
    return {n: np.asarray(o) for n, o in zip(out_names, outs)}


_NC = None


def kernel(**inputs):
    """Full-input entry point: shard, run on 8 NeuronCores, gather."""
    global _NC
    from concourse._compat import axon_active
    in_maps = prep_in_maps(inputs)
    if axon_active():
        return run_on_device(in_maps)["out"]
    from concourse.bass_utils import run_bass_kernel_spmd
    if _NC is None:
        _NC = build_nc(1)
    res = run_bass_kernel_spmd(_NC, in_maps, core_ids=list(range(N_CORES)))
    return np.concatenate([res.results[c]["out"] for c in range(N_CORES)],
                          axis=0)


# revision 14
# speedup vs baseline: 1.4502x; 1.1011x over previous
"""GRU unit kernel for Trainium2, data-parallel over 8 NeuronCores.

Computation (per batch row):
    r  = sigmoid(x @ W_i2r + b_i2r + h @ W_h2r)
    z  = sigmoid(x @ W_i2z + b_i2z + h @ W_h2z)
    h1 = tanh   (x @ W_i2h + b_i2h + r * (h @ W_h2h))
    out = (1 - z) * h1 + z * h

Sharding: batch (16384) split 8 ways; weights replicated.

Mixed-precision matmuls: per GEMM, the first F8[g] k-tiles (of 8 x 128)
are computed with fp8(e4m3) DoubleRow matmuls (2 k-tiles per pass, 2x PE
throughput), the rest with bf16 matmuls, all accumulating into the same
fp32 PSUM bank.  Inputs are pre-scaled host-side (x,h by 2^5; W by 2^12,
exact powers of two) so fp8 operands avoid the e4m3 subnormal range; the
2^-17 descale is folded into the ACT sigmoid/tanh `scale=` and the biases
are pre-scaled by 2^17.

Device kernel per core (B_local=2048 rows = 16 m-tiles of 128):
  - weights + x/h operands resident in SBUF, h (f32) and out streamed.
  - per m-tile: 8 PSUM banks (r,z,a,b x 2 N-halves of 512); stationary =
    activation tile, moving = weights; fp8 groups then bf16 groups per
    side; h-side ordered so r/b banks complete first and z last, letting
    the epilogue critical chain overlap the tail matmuls.
  - DMA rings: x-side bulk on sync, h-side bulk + h32 on gpsimd,
    outputs on scalar.
"""

import os
import numpy as np
import ml_dtypes
from contextlib import ExitStack

import concourse.bass as bass
import concourse.tile as tile
from concourse import bacc, mybir

if os.environ.get("GRU_LDWOPT", "0") == "1":
    import concourse.bass_utils as _bu
    if not getattr(_bu, "_gru_ldwopt_patched", False):
        _orig_run_command = _bu.run_command

        def _run_command_ldwopt(argv, **kwargs):
            argv = ["--enable-ldw-opt=true" if a == "--enable-ldw-opt=false"
                    else a for a in argv]
            return _orig_run_command(argv, **kwargs)

        _bu.run_command = _run_command_ldwopt
        _bu._gru_ldwopt_patched = True

N_CORES = 8
B, I, H = 16384, 1024, 1024
BL = B // N_CORES           # 2048 batch rows per core
MT = BL // 128              # 16 m-tiles
KO = I // 128               # 8 k-tiles of 128
F32 = mybir.dt.float32
BF16 = mybir.dt.bfloat16
FP8 = mybir.dt.float8e4
BF16_NP = ml_dtypes.bfloat16
FP8_NP = ml_dtypes.float8_e4m3
DR = mybir.MatmulPerfMode.DoubleRow

# fp8 k-tiles (even, 0..8) per GEMM: rx = x@W_i2r etc.
F8 = dict(
    rx=int(os.environ.get("GRU_F8_RX", "8")),
    rh=int(os.environ.get("GRU_F8_RH", "8")),
    zx=int(os.environ.get("GRU_F8_ZX", "4")),
    zh=int(os.environ.get("GRU_F8_ZH", "2")),
    ax=int(os.environ.get("GRU_F8_AX", "2")),
    bh=int(os.environ.get("GRU_F8_BH", "6")),
)

EPOFF = os.environ.get("GRU_EPOFF", "0") == "1"   # timing-only: no epilogue
KPORD = os.environ.get("GRU_KPORD", "1") == "1"   # r-only kp groups first

SX = 32.0                   # activation pre-scale (2^5)
SW = 4096.0                 # weight pre-scale (2^12)
ALPHA = 1.0 / (SX * SW)     # PSUM descale (2^-17)

XGATES = (("rx", 0), ("zx", 1), ("ax", 2))   # (key, column block in wx8)
HGATES = (("rh", 0), ("zh", 1), ("bh", 2))
XBANK = {"rx": "r", "zx": "z", "ax": "a"}
HBANK = {"rh": "r", "zh": "z", "bh": "b"}


def _b0(keys):
    need = [F8[k] for k in keys if F8[k] < KO]
    return min(need) if need else KO

KXB0 = _b0(["rx", "zx", "ax"])  # first k-tile with any bf16 x activation
KHB0 = _b0(["rh", "zh", "bh"])


def _ap_key(a):
    try:
        return (a.memref, a.offset, str(a.ap), str(a.dtype))
    except Exception:
        return ("?", id(a))


def dedupe_ldweights(nc):
    """Drop InstLdweights that reload the stationary tile already resident in
    the PE array (bacc emits one per matmul).  The paired InstMatmult keeps
    both APs, so data deps survive; the removed LDW's scheduling deps are
    merged into the following instruction."""
    total_removed = 0
    for blk in nc.m.functions[0].blocks:
        insts = list(blk.instructions)
        new = []
        last_key = None
        pending = []
        for i in insts:
            t = type(i).__name__
            eng = str(getattr(i, "engine", ""))
            if t == "InstLdweights":
                key = (_ap_key(i.ins[0]), str(i.perf_mode),
                       str(i.tile_position), str(i.is_transpose))
                if key == last_key:
                    pending.append(i)
                    total_removed += 1
                    continue
                last_key = key
                new.append(i)
            else:
                if "PE" in eng and t not in ("InstMatmult",
                                             "InstEventSemaphore"):
                    last_key = None  # unknown PE inst may clobber weights
                if pending and t == "InstMatmult":
                    for j in pending:
                        i.merge_dependencies_from(j)
                    pending = []
                new.append(i)
        if pending:
            new.extend(pending)
        blk.instructions = new
    return total_removed


def build_nc(reps: int = 1):
    nc = bacc.Bacc("TRN2", target_bir_lowering=False, debug=False,
                   num_devices=N_CORES)
    AF = mybir.ActivationFunctionType

    NXB = KO - KXB0             # bf16 k-tiles resident for x side
    NHB = KO - KHB0

    x8 = nc.dram_tensor("x8", [I, BL], FP8, kind="ExternalInput").ap()
    h8 = nc.dram_tensor("h8", [H, BL], FP8, kind="ExternalInput").ap()
    h32 = nc.dram_tensor("h32", [BL, H], F32, kind="ExternalInput").ap()
    wx8 = nc.dram_tensor("wx8", [I, 3 * H], FP8, kind="ExternalInput").ap()
    wh8 = nc.dram_tensor("wh8", [H, 3 * H], FP8, kind="ExternalInput").ap()
    bias = nc.dram_tensor("bias", [128, 3 * H], F32, kind="ExternalInput").ap()
    out = nc.dram_tensor("out", [BL, H], F32, kind="ExternalOutput").ap()

    # per-gate bf16 hi-k weights (exact ranges, no waste)
    wb_dram = {}
    for g in ("zx", "ax", "zh", "bh"):
        if F8[g] < KO:
            wb_dram[g] = nc.dram_tensor(
                f"wb_{g}", [(KO - F8[g]) * 128, H], BF16,
                kind="ExternalInput").ap()
    if NXB:
        xb = nc.dram_tensor("xb", [NXB * 128, BL], BF16,
                            kind="ExternalInput").ap()
    if NHB:
        hb = nc.dram_tensor("hb", [NHB * 128, BL], BF16,
                            kind="ExternalInput").ap()

    with tile.TileContext(nc) as tc, ExitStack() as ctx:
        wpool = ctx.enter_context(tc.tile_pool(name="w", bufs=1))
        apool = ctx.enter_context(tc.tile_pool(name="a", bufs=1))
        hpool = ctx.enter_context(tc.tile_pool(name="h", bufs=3))
        epool = ctx.enter_context(tc.tile_pool(name="e", bufs=2))
        psum = ctx.enter_context(tc.tile_pool(name="ps", bufs=1, space="PSUM"))

        wx8_sb = wpool.tile([128, KO, 3 * H], FP8, tag="wx8")
        wh8_sb = wpool.tile([128, KO, 3 * H], FP8, tag="wh8")
        bias_sb = wpool.tile([128, 3 * H], F32, tag="bias")
        x8_sb = apool.tile([128, KO, BL], FP8, tag="x8")
        h8_sb = apool.tile([128, KO, BL], FP8, tag="h8")
        wb_sb = {}
        for g in wb_dram:
            wb_sb[g] = wpool.tile([128, KO - F8[g], H], BF16, tag=f"wb{g}",
                                  name=f"wb{g}")
        if NXB:
            xb_sb = apool.tile([128, NXB, BL], BF16, tag="xb")
        if NHB:
            hb_sb = apool.tile([128, NHB, BL], BF16, tag="hb")

        x8_r = x8.rearrange("(ko ki) b -> ki ko b", ki=128)
        h8_r = h8.rearrange("(ko ki) b -> ki ko b", ki=128)
        wx8_r = wx8.rearrange("(ko ki) n -> ki ko n", ki=128)
        wh8_r = wh8.rearrange("(ko ki) n -> ki ko n", ki=128)
        wb_r = {g: wb_dram[g].rearrange("(ko ki) n -> ki ko n", ki=128)
                for g in wb_dram}
        if NXB:
            xb_r = xb.rearrange("(ko ki) b -> ki ko b", ki=128)
        if NHB:
            hb_r = hb.rearrange("(ko ki) b -> ki ko b", ki=128)

        h32_t = [None] * MT

        def h32_load(mt):
            if EPOFF:
                return
            if h32_t[mt] is None:
                h32_t[mt] = hpool.tile([128, H], F32, tag="h32", name="h32")
                nc.gpsimd.dma_start(h32_t[mt][:],
                                    h32[mt * 128:(mt + 1) * 128, :])

        def body():
            emit_loads()
            for mt in range(MT):
                emit_mtile(mt)
                h32_t[mt] = None

        def emit_loads():
            # FIFO order per ring: chunk-0 activations first (in the reps
            # loop these unblock ~25% into the previous iteration, so they
            # transfer early), then weights (blocked until the previous
            # iteration's last m-tile releases them), then the remaining
            # chunks.
            CH = 4
            cw = BL // CH
            c0 = slice(0, cw)
            nc.sync.dma_start(x8_sb[:, :, c0], x8_r[:, :, c0])
            if NXB:
                nc.sync.dma_start(xb_sb[:, :, c0], xb_r[:, :, c0])
            nc.gpsimd.dma_start(h8_sb[:, :, c0], h8_r[:, :, c0])
            if NHB:
                nc.gpsimd.dma_start(hb_sb[:, :, c0], hb_r[:, :, c0])
            for mt in range(3):
                h32_load(mt)
            for g, gi in XGATES:
                if F8[g]:
                    nc.sync.dma_start(
                        wx8_sb[:, 0:F8[g], gi * H:(gi + 1) * H],
                        wx8_r[:, 0:F8[g], gi * H:(gi + 1) * H])
            for g in ("zx", "ax"):
                if g in wb_sb:
                    nc.sync.dma_start(wb_sb[g][:], wb_r[g])
            nc.sync.dma_start(bias_sb[:], bias)
            for g, gi in HGATES:
                if F8[g]:
                    nc.gpsimd.dma_start(
                        wh8_sb[:, 0:F8[g], gi * H:(gi + 1) * H],
                        wh8_r[:, 0:F8[g], gi * H:(gi + 1) * H])
            for g in ("bh", "zh"):
                if g in wb_sb:
                    nc.gpsimd.dma_start(wb_sb[g][:], wb_r[g])
            for c in range(1, CH):
                cs = slice(c * cw, (c + 1) * cw)
                nc.sync.dma_start(x8_sb[:, :, cs], x8_r[:, :, cs])
                nc.gpsimd.dma_start(h8_sb[:, :, cs], h8_r[:, :, cs])
                if NXB:
                    nc.sync.dma_start(xb_sb[:, :, cs], xb_r[:, :, cs])
                if NHB:
                    nc.gpsimd.dma_start(hb_sb[:, :, cs], hb_r[:, :, cs])
                for mt in range(4 * c - 1, 4 * c + 3):
                    h32_load(mt)
            for mt in range(4 * CH - 1, MT):
                h32_load(mt)

        def emit_mtile(mt):
            ms = slice(mt * 128, (mt + 1) * 128)
            h32_load(mt)

            ps = {}
            for g in ("r", "z", "a", "b"):
                for nh in range(2):
                    ps[(g, nh)] = psum.tile([128, 512], F32, tag=f"p{g}{nh}",
                                            name=f"p{g}{nh}")
            started = set()

            def _passes(key):
                return F8[key] // 2 + (KO - F8[key])

            left = {
                "r": _passes("rx") + _passes("rh"),
                "z": _passes("zx") + _passes("zh"),
                "a": _passes("ax"),
                "b": _passes("bh"),
            }
            rem = {(g, nh): left[g] for g in left for nh in range(2)}

            def mm(bank, nh, stat, mov, perf_mode):
                key = (bank, nh)
                start = key not in started
                started.add(key)
                rem[key] -= 1
                nc.tensor.matmul(ps[key], stat, mov, start=start,
                                 stop=(rem[key] == 0), perf_mode=perf_mode)

            # ---- x side: fp8 DoubleRow groups (stationary = x8 tile) ----
            # r-only kp groups first: the m-tile's first matmuls then touch
            # only the r bank, which the previous epilogue frees earliest.
            kps = list(range(0, max(F8[g] for g, _ in XGATES), 2))
            if KPORD:
                kps.sort(key=lambda kp: sum(1 for g, _ in XGATES[1:]
                                            if F8[g] > kp))
            for kp in kps:
                stat = x8_sb[:, kp:kp + 2, ms]
                for nh in range(2):
                    o = nh * 512
                    for g, gi in XGATES:
                        if F8[g] > kp:
                            mm(XBANK[g], nh, stat,
                               wx8_sb[:, kp:kp + 2,
                                      gi * H + o:gi * H + o + 512], DR)
            # ---- x side: bf16 groups ----
            for ko in range(KXB0, KO):
                stat = xb_sb[:, ko - KXB0, ms]
                for nh in range(2):
                    o = nh * 512
                    for g in ("zx", "ax"):
                        if F8[g] <= ko:
                            mm(XBANK[g], nh, stat,
                               wb_sb[g][:, ko - F8[g], o:o + 512], None)
            # ---- h side: fp8 groups ----
            for kp in range(0, max(F8[g] for g, _ in HGATES), 2):
                stat = h8_sb[:, kp:kp + 2, ms]
                for nh in range(2):
                    o = nh * 512
                    for g, gi in HGATES:
                        if F8[g] > kp:
                            mm(HBANK[g], nh, stat,
                               wh8_sb[:, kp:kp + 2,
                                      gi * H + o:gi * H + o + 512], DR)
            # ---- h side: bf16 groups, b's k-tiles before z's ----
            for g in ("bh", "zh"):
                for ko in range(F8[g], KO):
                    stat = hb_sb[:, ko - KHB0, ms]
                    for nh in range(2):
                        o = nh * 512
                        mm(HBANK[g], nh, stat,
                           wb_sb[g][:, ko - F8[g], o:o + 512], None)

            if EPOFF:
                for nh in range(2):
                    o = nh * 512
                    sc = epool.tile([128, 512], F32, tag="tr", name="sc")
                    for g in ("r", "z", "a", "b"):
                        nc.vector.tensor_copy(sc[:], ps[(g, nh)][:])
                    nc.scalar.dma_start(out[ms, o:o + 512], sc[:])
                return

            # ---- epilogue ----
            for nh in range(2):
                o = nh * 512
                nsl = slice(o, o + 512)
                pr, pz = ps[("r", nh)], ps[("z", nh)]
                pa, pb = ps[("a", nh)], ps[("b", nh)]
                tr = epool.tile([128, 512], F32, tag="tr")
                tz = epool.tile([128, 512], F32, tag="tz")
                ta = epool.tile([128, 512], F32, tag="ta")
                nc.vector.tensor_add(tr[:], pr[:], bias_sb[:, o:o + 512])
                nc.scalar.activation(tr[:], tr[:], AF.Sigmoid, scale=ALPHA)
                nc.vector.tensor_mul(tr[:], tr[:], pb[:])            # r*(hU)
                nc.vector.tensor_add(ta[:], pa[:],
                                     bias_sb[:, 2 * H + o:2 * H + o + 512])
                nc.vector.tensor_add(ta[:], ta[:], tr[:])
                nc.scalar.activation(ta[:], ta[:], AF.Tanh, scale=ALPHA)
                nc.vector.tensor_add(tz[:], pz[:],
                                     bias_sb[:, H + o:H + o + 512])
                nc.scalar.activation(tz[:], tz[:], AF.Sigmoid, scale=ALPHA)
                nc.vector.tensor_sub(tr[:], h32_t[mt][:, nsl], ta[:])
                nc.vector.tensor_mul(tr[:], tz[:], tr[:])            # z*(h-h1)
                nc.vector.tensor_add(tr[:], ta[:], tr[:])            # out
                nc.scalar.dma_start(out[ms, nsl], tr[:])

        if reps > 1:
            with tc.For_i(0, reps, 1):
                body()
        else:
            body()

    nc.compile()
    if os.environ.get("GRU_DEDUP", "1") == "1":
        dedupe_ldweights(nc)
    return nc


def prep_in_maps(inputs):
    """Host-side marshalling: shard batch, transpose/scale/quantize
    activations, concat weights/biases.  Returns per-core input dicts."""
    g = {k: np.asarray(v) for k, v in inputs.items()}
    x, h = g["inputs"].astype(np.float32), g["hidden"].astype(np.float32)
    wx = np.concatenate([g["W_i2r"], g["W_i2z"], g["W_i2h"]],
                        axis=1).astype(np.float32) * SW
    wh = np.concatenate([g["W_h2r"], g["W_h2z"], g["W_h2h"]],
                        axis=1).astype(np.float32) * SW
    b = np.concatenate([g["b_i2r"], g["b_i2z"], g["b_i2h"]]).astype(np.float32)
    bias_b = np.ascontiguousarray(np.broadcast_to(b / ALPHA, (128, 3 * H)))

    xs = np.ascontiguousarray(x.T) * SX          # [I, B], scaled
    hs = np.ascontiguousarray(h.T) * SX
    x8_all = xs.astype(FP8_NP)
    h8_all = hs.astype(FP8_NP)
    wx8 = np.ascontiguousarray(wx).astype(FP8_NP)
    wh8 = np.ascontiguousarray(wh).astype(FP8_NP)

    wcol = {"zx": wx[:, H:2 * H], "ax": wx[:, 2 * H:3 * H],
            "zh": wh[:, H:2 * H], "bh": wh[:, 2 * H:3 * H]}
    wb = {}
    for gkey in ("zx", "ax", "zh", "bh"):
        if F8[gkey] < KO:
            wb[gkey] = np.ascontiguousarray(
                wcol[gkey][F8[gkey] * 128:]).astype(BF16_NP)

    NXB = KO - KXB0
    NHB = KO - KHB0
    if NXB:
        xb_all = np.ascontiguousarray(xs[KXB0 * 128:]).astype(BF16_NP)
    if NHB:
        hb_all = np.ascontiguousarray(hs[KHB0 * 128:]).astype(BF16_NP)

    in_maps = []
    for c in range(N_CORES):
        sl = slice(c * BL, (c + 1) * BL)
        m = {
            "x8": np.ascontiguousarray(x8_all[:, sl]),
            "h8": np.ascontiguousarray(h8_all[:, sl]),
            "h32": np.ascontiguousarray(h[sl]),
            "wx8": wx8,
            "wh8": wh8,
            "bias": bias_b,
        }
        for gkey, arr in wb.items():
            m[f"wb_{gkey}"] = arr
        if NXB:
            m["xb"] = np.ascontiguousarray(xb_all[:, sl])
        if NHB:
            m["hb"] = np.ascontiguousarray(hb_all[:, sl])
        in_maps.append(m)
    return in_maps


_RUNNERS = {}


def get_runner(reps: int = 1):
    """Build the bass module once and wrap it in a jitted 8-way shard_map
    (so repeated executions don't re-trace/re-compile).  reps>1 wraps the
    whole kernel in an on-device loop (for timing via amortization)."""
    if reps in _RUNNERS:
        return _RUNNERS[reps]
    import jax
    from jax.sharding import Mesh, PartitionSpec
    from jax.experimental.shard_map import shard_map
    from concourse.bass2jax import (_bass_exec_p, install_neuronx_cc_hook,
                                    partition_id_tensor)

    nc = build_nc(reps)
    install_neuronx_cc_hook()

    partition_name = (nc.partition_id_tensor.name
                      if nc.partition_id_tensor else None)
    in_names, out_names, out_avals, zero_outs = [], [], [], []
    for alloc in nc.m.functions[0].allocations:
        if not isinstance(alloc, mybir.MemoryLocationSet):
            continue
        name = alloc.memorylocations[0].name
        if alloc.kind == "ExternalInput":
            if name != partition_name:
                in_names.append(name)
        elif alloc.kind == "ExternalOutput":
            out_names.append(name)
            shape = tuple(alloc.tensor_shape)
            dtype = mybir.dt.np(alloc.dtype)
            out_avals.append(jax.core.ShapedArray(shape, dtype))
            zero_outs.append(np.zeros(shape, dtype))
    all_names = in_names + out_names
    if partition_name is not None:
        all_names = all_names + [partition_name]
    all_names = tuple(all_names)
    n_in, n_out = len(in_names), len(out_names)

    def _body(*args):
        operands = list(args)
        if partition_name is not None:
            operands.append(partition_id_tensor())
        outs = _bass_exec_p.bind(
            *operands,
            out_avals=tuple(out_avals),
            in_names=all_names,
            out_names=tuple(out_names),
            lowering_input_output_aliases=(),
            sim_require_finite=True,
            sim_require_nnan=True,
            nc=nc,
        )
        return tuple(outs)

    devices = jax.devices()[:N_CORES]
    mesh = Mesh(np.asarray(devices), ("core",))
    sharded = jax.jit(
        shard_map(_body, mesh=mesh,
                  in_specs=(PartitionSpec("core"),) * (n_in + n_out),
                  out_specs=(PartitionSpec("core"),) * n_out,
                  check_rep=False),
        donate_argnums=tuple(range(n_in, n_in + n_out)),
        keep_unused=True,
    )
    _RUNNERS[reps] = (sharded, in_names, out_names, zero_outs)
    return _RUNNERS[reps]


def run_on_device(in_maps):
    sharded, in_names, out_names, zero_outs = get_runner()
    concat_in = [np.concatenate([m[n] for m in in_maps], axis=0)
                 for n in in_names]
    concat_zero = [np.zeros((N_CORES * z.shape[0], *z.shape[1:]), z.dtype)
                   for z in zero_outs]
    outs = sharded(*concat_in, *concat_zero)
    return {n: np.asarray(o) for n, o in zip(out_names, outs)}


_NC = None


def kernel(**inputs):
    """Full-input entry point: shard, run on 8 NeuronCores, gather."""
    global _NC
    from concourse._compat import axon_active
    in_maps = prep_in_maps(inputs)
    if axon_active():
        return run_on_device(in_maps)["out"]
    from concourse.bass_utils import run_bass_kernel_spmd
    if _NC is None:
        _NC = build_nc(1)
    res = run_bass_kernel_spmd(_NC, in_maps, core_ids=list(range(N_CORES)))
    return np.concatenate([res.results[c]["out"] for c in range(N_CORES)],
                          axis=0)
